# revision 24
# baseline (speedup 1.0000x reference)
"""Trainium2 Bass kernel for nn_DeepUDI (RGAT+GRU message passing), 8-core SPMD.

Sharding: nodes (dim 0) split across 8 cores; 256 nodes = 512 (node,relation)
pairs per core. Neighbor gather + weight folding on host (graph-parallel, no
collectives), all per-pair math on device.

Host-side algebraic folds (exact in fp32, weights-only):
  A    = w @ kw @ qw^T @ w^T        (attention scores = hn . (A h))
  WnW0 = w @ Wn0,  WnW1 = w @ Wn1   (gate pre-acts act on g = hn^T softmax(E))

Device structure (all-fp16 operands, fp32 PSUM accumulation):
Pairs are PSUM columns. Per-pair matvecs are packed so each TensorE
LDWEIGHTS+MATMUL covers 2 pairs (4 for the 32-row hn stationary): stationaries
stack two pairs' matrices vertically in the 128 partitions, and the moving
operand holds block-"diagonal" columns ([v_e;0], [0;v_o]) so one matmul with
N=2 computes both pairs without cross-terms. This removes the fp32 2-pass
matmul penalty, halves score-path HBM bytes, and ~halves TensorE instructions
vs per-pair N=1 matmuls.

Stages per tile (TS pairs; TS=64 first/last for pipeline ramp, 128 middle):
  A : u = A h            lhsT=[A_e^T;A_o^T](128x64)       rhs=h-diag   N=2
  C : scores = hn u      lhsT=[hn_e^T;hn_o^T](128x32)     rhs=u-diag   N=2
  softmax over K=32 (exp on ACT, sums/broadcast via ones-matmuls,
  sigmoid via tanh-form to avoid ACT table thrash)
  D : g = hn^T E         lhsT=[hn_0;..;hn_3](128x64)      rhs=E-diag   N=4
  RZ: [Rpre;Zpre]        lhsT=[[Wx0;WnW0]|[Wx1;WnW1]]     rhs=[h;g]    N=1
  W : df = w^T g         lhsT=[w_e;w_o](128x64)           rhs=g-diag   N=2
  H : Hpre               lhsT=[SH_e|SH_o](128x128)        rhs=[h;rdf]  N=2
      (SH=[Wx2;Wn2]; diag output rows 0:64 even cols / 64:128 odd cols)
  gru = Z df + (1-Z) tanh(Hpre+b2);  out = tanh(mean_r gru)

DMA: two queues — sync-HWDGE carries tensors whose SBUF slots free early
(srz has 3 bufs); gpsimd-SWDGE carries late-released tensors so their slot
stalls cannot head-of-line-block the main stream.
"""

import numpy as np

N, R, K, D, F = 2048, 2, 32, 64, 64
P_ALL = N * R            # 4096 pairs
NCORES = 8
PPC = P_ALL // NCORES    # 512 pairs/core
NPC = N // NCORES        # 256 nodes/core
TSIZES = [64, 128, 128, 128, 32, 32]   # pairs per tile (sum = PPC)
TMAX = max(TSIZES)
NT = len(TSIZES)

_cache = {}


def _build():
    import concourse.mybir as mybir
    import concourse.tile as tile
    from concourse import bacc

    fp32 = mybir.dt.float32
    fp16 = mybir.dt.float16
    Tanh = mybir.ActivationFunctionType.Tanh
    Exp = mybir.ActivationFunctionType.Exp

    nc = bacc.Bacc(
        "TRN2", target_bir_lowering=False, debug=False, num_devices=NCORES
    )

    # ---- DRAM I/O (per-core shards; one tensor set per tile) ----
    dr = []
    for t, TS in enumerate(TSIZES):
        u2, u4 = TS // 2, TS // 4
        dr.append({
            "SA": nc.dram_tensor(f"SA{t}", [128, u2 * D], fp16, kind="ExternalInput"),
            "SC": nc.dram_tensor(f"SC{t}", [128, u2 * K], fp16, kind="ExternalInput"),
            "SD": nc.dram_tensor(f"SD{t}", [128, u4 * D], fp16, kind="ExternalInput"),
            "SRZ": nc.dram_tensor(f"SRZ{t}", [128, TS * 2 * F], fp16, kind="ExternalInput"),
            "SW": nc.dram_tensor(f"SW{t}", [128, u2 * F], fp16, kind="ExternalInput"),
            "SH": nc.dram_tensor(f"SH{t}", [128, u2 * 2 * F], fp16, kind="ExternalInput"),
            "HD": nc.dram_tensor(f"HD{t}", [128, TS], fp16, kind="ExternalInput"),
            "Hb": nc.dram_tensor(f"Hb{t}", [D, TS], fp16, kind="ExternalInput"),
            "BB": nc.dram_tensor(f"BB{t}", [128, 2 * TS], fp32, kind="ExternalInput"),
            "out": nc.dram_tensor(f"out{t}", [F, u2], fp32, kind="ExternalOutput"),
        })

    with tile.TileContext(nc) as tc:
        with (
            tc.tile_pool(name="const", bufs=1) as cpool,
            tc.tile_pool(name="stat", bufs=2) as spool,
            tc.tile_pool(name="big", bufs=3) as bpool,
            tc.tile_pool(name="mv", bufs=4) as mvpool,
            tc.tile_pool(name="vec", bufs=2) as vpool,
            tc.tile_pool(name="pA", bufs=2, space="PSUM") as pA_pool,
            tc.tile_pool(name="pM", bufs=2, space="PSUM") as pM_pool,
            tc.tile_pool(name="pGDF", bufs=2, space="PSUM") as pGDF_pool,
            tc.tile_pool(name="pRZ", bufs=1, space="PSUM") as pRZ_pool,
            tc.tile_pool(name="pH", bufs=1, space="PSUM") as pH_pool,
        ):
            ones_k1 = cpool.tile([K, 1], fp32)
            nc.vector.memset(ones_k1, 1.0)
            ones_1k = cpool.tile([1, K], fp32)
            nc.vector.memset(ones_1k, 1.0)

            for t, TS in enumerate(TSIZES):
                u2, u4 = TS // 2, TS // 4
                dt = dr[t]
                # sync-HWDGE queue: early-released / spare-buffered tensors.
                srz = bpool.tile([128, TMAX * 2 * F], fp16, tag="srz")
                nc.sync.dma_start(out=srz[:, 0 : TS * 2 * F], in_=dt["SRZ"][:, :])
                sa = spool.tile([128, (TMAX // 2) * D], fp16, tag="sa")
                nc.sync.dma_start(out=sa[:, 0 : u2 * D], in_=dt["SA"][:, :])
                sc = spool.tile([128, (TMAX // 2) * K], fp16, tag="sc")
                nc.sync.dma_start(out=sc[:, 0 : u2 * K], in_=dt["SC"][:, :])
                sd = spool.tile([128, (TMAX // 4) * D], fp16, tag="sd")
                nc.sync.dma_start(out=sd[:, 0 : u4 * D], in_=dt["SD"][:, :])
                hd = mvpool.tile([128, TMAX], fp16, tag="hd")
                nc.sync.dma_start(out=hd[:, 0:TS], in_=dt["HD"][:, :])
                # gpsimd-SWDGE queue: late-released tensors.
                dq = nc.sync if t == 0 else nc.gpsimd
                sw = spool.tile([128, (TMAX // 2) * F], fp16, tag="sw")
                dq.dma_start(out=sw[:, 0 : u2 * F], in_=dt["SW"][:, :])
                sh = spool.tile([128, (TMAX // 2) * 2 * F], fp16, tag="sh")
                dq.dma_start(out=sh[:, 0 : u2 * 2 * F], in_=dt["SH"][:, :])
                hg = mvpool.tile([128, TMAX], fp16, tag="hg")
                dq.dma_start(out=hg[0:D, 0:TS], in_=dt["Hb"][:, :])
                hrdf = mvpool.tile([128, TMAX], fp16, tag="hrdf")
                dq.dma_start(out=hrdf[0:D, 0:TS], in_=dt["Hb"][:, :])
                bb = mvpool.tile([128, 2 * TMAX], fp32, tag="bb")
                dq.dma_start(out=bb[:, 0 : 2 * TS], in_=dt["BB"][:, :])

                # ---- stage A: u = A h ----
                pA = pA_pool.tile([D, TMAX], fp32, tag="pA")
                for i in range(u2):
                    nc.tensor.matmul(
                        pA[:, 2 * i : 2 * i + 2],
                        sa[:, i * D : (i + 1) * D],
                        hd[:, 2 * i : 2 * i + 2],
                        start=True, stop=True,
                    )
                # u-diag restack: col 2i = [u_e;0], col 2i+1 = [0;u_o]
                uc = vpool.tile([128, TMAX], fp16, tag="uc")
                if t < 2:
                    nc.vector.memset(uc, 0.0)
                pA_v = pA[:, 0:TS].rearrange("d (u r) -> d r u", r=2)
                nc.vector.tensor_copy(
                    uc[0:D, 0:TS].rearrange("d (u r) -> d r u", r=2)[:, 0, :],
                    pA_v[:, 0, :],
                )
                nc.vector.tensor_copy(
                    uc[D:128, 0:TS].rearrange("d (u r) -> d r u", r=2)[:, 1, :],
                    pA_v[:, 1, :],
                )

                # ---- stage C: scores = hn u ----
                pM = pM_pool.tile([K, 3 * TMAX], fp32, tag="pM")
                psc = pM[:, 0:TS]
                for i in range(u2):
                    nc.tensor.matmul(
                        psc[:, 2 * i : 2 * i + 2],
                        sc[:, i * K : (i + 1) * K],
                        uc[:, 2 * i : 2 * i + 2],
                        start=True, stop=True,
                    )
                # softmax over K: E = exp(scores); Ehat = E / sum_k E
                E_sb = vpool.tile([K, TMAX], fp32, tag="esb")
                nc.scalar.activation(out=E_sb[:, 0:TS], in_=psc, func=Exp)
                ps_row = pM[0:1, TMAX : TMAX + TS]
                nc.tensor.matmul(ps_row, ones_k1, E_sb[:, 0:TS], start=True, stop=True)
                rs_sb = vpool.tile([1, TMAX], fp32, tag="rssb")
                nc.vector.reciprocal(rs_sb[:, 0:TS], ps_row)
                prsb = pM[:, 2 * TMAX : 2 * TMAX + TS]
                nc.tensor.matmul(prsb, ones_1k, rs_sb[:, 0:TS], start=True, stop=True)
                ehat = vpool.tile([K, TMAX], fp32, tag="ehat")
                nc.vector.tensor_tensor(
                    out=ehat[:, 0:TS], in0=E_sb[:, 0:TS], in1=prsb,
                    op=mybir.AluOpType.mult,
                )
                # E-diag restack: col 4i+j has E at rows 32j:32j+32
                ed = vpool.tile([128, TMAX], fp16, tag="ed")
                if t < 2:
                    nc.vector.memset(ed, 0.0)
                ehat_v = ehat[:, 0:TS].rearrange("k (u r) -> k r u", r=4)
                for j in range(4):
                    nc.vector.tensor_copy(
                        ed[32 * j : 32 * j + 32, 0:TS]
                        .rearrange("k (u r) -> k r u", r=4)[:, j, :],
                        ehat_v[:, j, :],
                    )

                # ---- stage D: g = hn^T Ehat ----
                pG = pGDF_pool.tile([D, TMAX], fp32, tag="pgdf")
                for i in range(u4):
                    nc.tensor.matmul(
                        pG[:, 4 * i : 4 * i + 4],
                        sd[:, i * D : (i + 1) * D],
                        ed[:, 4 * i : 4 * i + 4],
                        start=True, stop=True,
                    )
                # hg rows D:128 = g (straight); g-diag restack for stage W
                nc.vector.tensor_copy(hg[D:128, 0:TS], pG[:, 0:TS])
                gd = vpool.tile([128, TMAX], fp16, tag="gd")
                if t < 2:
                    nc.vector.memset(gd, 0.0)
                pG_v = pG[:, 0:TS].rearrange("d (u r) -> d r u", r=2)
                nc.vector.tensor_copy(
                    gd[0:D, 0:TS].rearrange("d (u r) -> d r u", r=2)[:, 0, :],
                    pG_v[:, 0, :],
                )
                nc.vector.tensor_copy(
                    gd[D:128, 0:TS].rearrange("d (u r) -> d r u", r=2)[:, 1, :],
                    pG_v[:, 1, :],
                )

                # ---- stage RZ: [Rpre;Zpre] = [[Wx0;WnW0]|[Wx1;WnW1]]^T [h;g] ----
                pRZ = pRZ_pool.tile([128, TMAX], fp32, tag="pRZ")
                for p in range(TS):
                    nc.tensor.matmul(
                        pRZ[:, p : p + 1],
                        srz[:, p * 2 * F : (p + 1) * 2 * F],
                        hg[:, p : p + 1],
                        start=True, stop=True,
                    )
                RZp = vpool.tile([128, TMAX], fp32, tag="RZp")
                nc.vector.tensor_add(RZp[:, 0:TS], pRZ[:, 0:TS], bb[:, 0:TS])
                # sigmoid(x) = 0.5 + 0.5*tanh(x/2): keeps ACT table on {Exp,Tanh}
                RZt = vpool.tile([128, TMAX], fp32, tag="RZt")
                nc.scalar.activation(
                    out=RZt[:, 0:TS], in_=RZp[:, 0:TS], func=Tanh, scale=0.5
                )
                RZs = vpool.tile([128, TMAX], fp32, tag="RZs")
                nc.vector.tensor_scalar(
                    out=RZs[:, 0:TS], in0=RZt[:, 0:TS], scalar1=0.5, scalar2=0.5,
                    op0=mybir.AluOpType.mult, op1=mybir.AluOpType.add,
                )


                # ---- stage W: df = w^T g ----
                pDF = pGDF_pool.tile([F, TMAX], fp32, tag="pgdf")
                for i in range(u2):
                    nc.tensor.matmul(
                        pDF[:, 2 * i : 2 * i + 2],
                        sw[:, i * F : (i + 1) * F],
                        gd[:, 2 * i : 2 * i + 2],
                        start=True, stop=True,
                    )
                # hrdf rows D:128 = sigmoid(Rpre) * df  (Rg lives at rows
                # D:128 of RZs because SRZ packs [Z|R]; same-base SBUF ops)
                nc.vector.tensor_tensor(
                    out=hrdf[D:128, 0:TS], in0=RZs[D:128, 0:TS],
                    in1=pDF[:, 0:TS], op=mybir.AluOpType.mult,
                )

                # ---- stage H: Hpre (diag out) ----
                pH = pH_pool.tile([128, TMAX], fp32, tag="pH")
                for i in range(u2):
                    nc.tensor.matmul(
                        pH[:, 2 * i : 2 * i + 2],
                        sh[:, i * 2 * F : (i + 1) * 2 * F],
                        hrdf[:, 2 * i : 2 * i + 2],
                        start=True, stop=True,
                    )
                tHp = vpool.tile([128, TMAX], fp32, tag="tHp")
                nc.vector.tensor_add(
                    tHp[:, 0:TS], pH[:, 0:TS], bb[:, TS : 2 * TS]
                )
                Hcd = vpool.tile([128, TMAX], fp32, tag="Hcd")
                nc.scalar.activation(out=Hcd[:, 0:TS], in_=tHp[:, 0:TS], func=Tanh)
                # assemble Hc straight [F, TS] from diag halves
                Hc = vpool.tile([F, TMAX], fp32, tag="Hc")
                Hc_v = Hc[:, 0:TS].rearrange("f (u r) -> f r u", r=2)
                nc.vector.tensor_copy(
                    Hc_v[:, 0, :],
                    Hcd[0:D, 0:TS].rearrange("f (u r) -> f r u", r=2)[:, 0, :],
                )
                nc.vector.tensor_copy(
                    Hc_v[:, 1, :],
                    Hcd[D:128, 0:TS].rearrange("f (u r) -> f r u", r=2)[:, 1, :],
                )

                # ---- gru = Hc + Z*(df - Hc); out = tanh(mean_r gru) ----
                gru = vpool.tile([F, TMAX], fp32, tag="gru")
                nc.vector.tensor_sub(gru[:, 0:TS], pDF[:, 0:TS], Hc[:, 0:TS])
                nc.vector.tensor_mul(gru[:, 0:TS], gru[:, 0:TS], RZs[0:D, 0:TS])
                nc.vector.tensor_add(gru[:, 0:TS], gru[:, 0:TS], Hc[:, 0:TS])
                tcol = vpool.tile([F, TMAX // 2], fp32, tag="tcol")
                gru_v = gru[:, 0:TS].rearrange("f (u r) -> f r u", r=2)
                nc.vector.tensor_add(tcol[:, 0:u2], gru_v[:, 0, :], gru_v[:, 1, :])
                osb = vpool.tile([F, TMAX // 2], fp32, tag="osb")
                nc.scalar.activation(
                    out=osb[:, 0:u2], in_=tcol[:, 0:u2], func=Tanh, scale=0.5
                )
                # scalar-HWDGE queue: ACT just produced osb, so this issues
                # with zero wait and cannot head-of-line-block input DMAs.
                nc.scalar.dma_start(out=dt["out"][:, :], in_=osb[:, 0:u2])

    nc.compile()
    return nc


def _prep(inputs):
    f16 = np.float16
    x = np.asarray(inputs["x"]).astype(np.int64)
    nbr = np.asarray(inputs["neighbors"]).astype(np.int64).reshape(P_ALL, K)
    embed = np.asarray(inputs["embed"], dtype=np.float32)
    w = np.asarray(inputs["w"], dtype=np.float32).reshape(P_ALL, D, F)
    qw = np.asarray(inputs["qw"], dtype=np.float32).reshape(P_ALL, F, -1)
    kw = np.asarray(inputs["kw"], dtype=np.float32).reshape(P_ALL, F, -1)
    Wx = np.asarray(inputs["Wx"], dtype=np.float32).reshape(P_ALL, 3, D, F)
    Wn = np.asarray(inputs["Wn"], dtype=np.float32).reshape(P_ALL, 3, F, F)
    b = (
        np.asarray(inputs["bx"], dtype=np.float32)
        + np.asarray(inputs["bn"], dtype=np.float32)
    ).reshape(P_ALL, 3, F)

    h = embed[x]                                    # [N, D]
    hp = np.repeat(h, R, axis=0)                    # [P, D] center node per pair
    hn = h[nbr]                                     # [P, K, D]
    A = w @ kw @ qw.transpose(0, 2, 1) @ w.transpose(0, 2, 1)   # [P, D, D]
    WnW0 = w @ Wn[:, 0]
    WnW1 = w @ Wn[:, 1]

    def stack2(M):
        # [TS, D, X] -> [128, (TS//2)*X]: rows r*64+d, cols u*X+j
        TS, _, X = M.shape
        return (
            M.reshape(TS // 2, 2, D, X)
            .transpose(1, 2, 0, 3)
            .reshape(128, (TS // 2) * X)
        )

    in_maps = []
    for c in range(NCORES):
        m = {}
        for t, TS in enumerate(TSIZES):
            off = c * PPC + sum(TSIZES[:t])
            s = slice(off, off + TS)
            u2 = TS // 2
            A_c, hn_c, w_c = A[s], hn[s], w[s]
            Wx_c, Wn2_c = Wx[s], Wn[s, 2]
            W0_c, W1_c = WnW0[s], WnW1[s]
            b_c, hp_c = b[s], hp[s]

            SA = stack2(A_c.transpose(0, 2, 1))                 # A^T blocks
            SC = stack2(hn_c.transpose(0, 2, 1))                # hn^T blocks
            SD = (
                hn_c.reshape(TS // 4, 4, K, D)
                .transpose(1, 2, 0, 3)
                .reshape(128, (TS // 4) * D)
            )
            RZblk = np.concatenate(
                [
                    np.concatenate([Wx_c[:, 1], W1_c], axis=1),  # Z gate first
                    np.concatenate([Wx_c[:, 0], W0_c], axis=1),  # R gate rows 64:
                ],
                axis=2,
            )                                                    # [TS, 128, 2F]
            SRZ = RZblk.transpose(1, 0, 2).reshape(128, TS * 2 * F)
            SW = stack2(w_c)
            SHp = np.concatenate([Wx_c[:, 2], Wn2_c], axis=1)    # [TS, 128, F]
            SH = (
                SHp.reshape(u2, 2, 128, F)
                .transpose(2, 0, 1, 3)
                .reshape(128, u2 * 2 * F)
            )
            Z = np.zeros((TS, 128), np.float32)
            Z[0::2, 0:D] = hp_c[0::2]
            Z[1::2, D:128] = hp_c[1::2]
            HD = Z.T
            Hb = hp_c.T                                          # [D, TS]
            B01 = np.concatenate([b_c[:, 1], b_c[:, 0]], axis=1).T  # [Z|R] [128, TS]
            Z2 = np.zeros((TS, 128), np.float32)
            Z2[0::2, 0:F] = b_c[0::2, 2]
            Z2[1::2, F:128] = b_c[1::2, 2]
            B2D = Z2.T
            BB = np.concatenate([B01, B2D], axis=1)              # [128, 2*TS]

            m[f"SA{t}"] = np.ascontiguousarray(SA).astype(f16)
            m[f"SC{t}"] = np.ascontiguousarray(SC).astype(f16)
            m[f"SD{t}"] = np.ascontiguousarray(SD).astype(f16)
            m[f"SRZ{t}"] = np.ascontiguousarray(SRZ).astype(f16)
            m[f"SW{t}"] = np.ascontiguousarray(SW).astype(f16)
            m[f"SH{t}"] = np.ascontiguousarray(SH).astype(f16)
            m[f"HD{t}"] = np.ascontiguousarray(HD).astype(f16)
            m[f"Hb{t}"] = np.ascontiguousarray(Hb).astype(f16)
            m[f"BB{t}"] = np.ascontiguousarray(BB)
        in_maps.append(m)
    return in_maps


def kernel(**inputs):
    from concourse.bass_utils import run_bass_kernel_spmd

    if "nc" not in _cache:
        _cache["nc"] = _build()
    in_maps = _prep(inputs)
    res = run_bass_kernel_spmd(_cache["nc"], in_maps, list(range(NCORES)))
    outs = []
    for c in range(NCORES):
        parts = [
            res.results[c][f"out{t}"].T.reshape(TSIZES[t] // 2, F)
            for t in range(NT)
        ]
        outs.append(np.concatenate(parts, axis=0))
    return np.concatenate(outs, axis=0).astype(np.float32)
